# revision 1
# baseline (speedup 1.0000x reference)
"""GNN message-passing kernel for TRN2, 8-core SPMD (self-contained).

Design:
- Node rows sharded 8 ways (NS=N/8 per core), edge rows too (ES=E/8).
- All gathers via gpsimd.dma_gather (int16 wrapped indices precomputed on the
  host). Padding (-1) indices are remapped host-side to row 0; the spurious
  row-0 contributions are cancelled by a rank-1 correction matmul whose
  per-row coefficients are precomputed on the host.
- Per 128-row block:
    * dma_gather neighbor node rows / incident edge rows -> SBUF [128, 8*D]
    * masked mean via accumulating PE matmuls psum += diag(1/cnt) @ G_j,
      plus rank-1 corrections corr (x) table_row0
    * PE transposes build X^T (feature-major) chunks
    * linear layer: psum_o += X^T_chunk.T @ W_chunk, bias via rank-1 matmul,
      ReLU fused in the psum->SBUF copy on the scalar engine
    * store node-major result to shard staging; AllGather to next table
"""
import sys

sys.path.insert(0, '/opt/trn_rl_repo')

import numpy as np
import concourse.bass as bass
import concourse.mybir as mybir
from concourse import tile
from concourse.bacc import Bacc
from concourse.masks import make_identity

F32 = mybir.dt.float32
I16 = mybir.dt.int16
P = 128


class Cfg:
    def __init__(self, N=8192, E=32768, D=512, DEG=16, DEP=8, K=3, CORES=8):
        self.N, self.E, self.D = N, E, D
        self.DEG, self.DEP, self.K, self.CORES = DEG, DEP, K, CORES
        self.NS = N // CORES
        self.ES = E // CORES
        self.NB = self.NS // P
        self.EB = self.ES // P
        self.DC = D // P          # feature chunks per D
        self.KCN = (2 * D) // P   # contraction chunks, node linear
        self.KCE = (3 * D) // P   # contraction chunks, edge linear
        assert self.NS % P == 0 and self.ES % P == 0 and D % P == 0
        assert N <= 32768 and E <= 32768  # int16 dma_gather indices


def build(cfg: Cfg, gbufs=9, table_bf16=False):
    N, E, D = cfg.N, cfg.E, cfg.D
    DEG, DEP, K, CORES = cfg.DEG, cfg.DEP, cfg.K, cfg.CORES
    NS, ES, NB, EB = cfg.NS, cfg.ES, cfg.NB, cfg.EB
    DC, KCN, KCE = cfg.DC, cfg.KCN, cfg.KCE
    # wrapped-index columns per block: (width*128)/16
    NCOL = DEG * 8
    ECOL = DEP * 8
    TDT = mybir.dt.bfloat16 if table_bf16 else F32

    nc = Bacc("TRN2", target_bir_lowering=False, debug=False, num_devices=CORES,
              num_swdge_queues=4)

    # ---- external inputs ----
    fw_tab0 = nc.dram_tensor("fw_tab0", [N, D], TDT, kind="ExternalInput")
    bw_tab0 = nc.dram_tensor("bw_tab0", [N, D], TDT, kind="ExternalInput")
    e_tab0 = nc.dram_tensor("e_tab0", [E, D], TDT, kind="ExternalInput")
    fw_own0 = nc.dram_tensor("fw_own0", [NS, D], F32, kind="ExternalInput")
    bw_own0 = nc.dram_tensor("bw_own0", [NS, D], F32, kind="ExternalInput")
    e_own0 = nc.dram_tensor("e_own0", [ES, D], F32, kind="ExternalInput")
    fw_adj_g = nc.dram_tensor("fw_adj_g", [P, NB * NCOL], I16, kind="ExternalInput")
    bw_adj_g = nc.dram_tensor("bw_adj_g", [P, NB * NCOL], I16, kind="ExternalInput")
    fw_eid_g = nc.dram_tensor("fw_eid_g", [P, NB * NCOL], I16, kind="ExternalInput")
    bw_eid_g = nc.dram_tensor("bw_eid_g", [P, NB * NCOL], I16, kind="ExternalInput")
    fw_dep_g = nc.dram_tensor("fw_dep_g", [P, EB * ECOL], I16, kind="ExternalInput")
    bw_dep_g = nc.dram_tensor("bw_dep_g", [P, EB * ECOL], I16, kind="ExternalInput")
    rcn_fw = nc.dram_tensor("rcn_fw", [NS, 1], F32, kind="ExternalInput")
    rcn_bw = nc.dram_tensor("rcn_bw", [NS, 1], F32, kind="ExternalInput")
    rce_fw = nc.dram_tensor("rce_fw", [ES, 1], F32, kind="ExternalInput")
    rce_bw = nc.dram_tensor("rce_bw", [ES, 1], F32, kind="ExternalInput")
    ca_fw = nc.dram_tensor("ca_fw", [NS, 1], F32, kind="ExternalInput")
    ca_bw = nc.dram_tensor("ca_bw", [NS, 1], F32, kind="ExternalInput")
    ce_fw = nc.dram_tensor("ce_fw", [NS, 1], F32, kind="ExternalInput")
    ce_bw = nc.dram_tensor("ce_bw", [NS, 1], F32, kind="ExternalInput")
    cd_fw = nc.dram_tensor("cd_fw", [ES, 1], F32, kind="ExternalInput")
    cd_bw = nc.dram_tensor("cd_bw", [ES, 1], F32, kind="ExternalInput")
    Wfc = nc.dram_tensor("Wfc", [2 * D, D], F32, kind="ExternalInput")
    Wbc = nc.dram_tensor("Wbc", [2 * D, D], F32, kind="ExternalInput")
    Wedge = nc.dram_tensor("Wedge", [3 * D, D], F32, kind="ExternalInput")
    bfc = nc.dram_tensor("bfc", [1, D], F32, kind="ExternalInput")
    bbc = nc.dram_tensor("bbc", [1, D], F32, kind="ExternalInput")
    bedge = nc.dram_tensor("bedge", [1, D], F32, kind="ExternalInput")
    fw_out = nc.dram_tensor("fw_out", [NS, D], F32, kind="ExternalOutput")
    bw_out = nc.dram_tensor("bw_out", [NS, D], F32, kind="ExternalOutput")

    rg = [list(range(CORES))]

    with tile.TileContext(nc) as tc:
        with (
            tc.tile_pool(name="const", bufs=1) as cp,
            tc.tile_pool(name="gp", bufs=gbufs) as gp,
            tc.tile_pool(name="ip", bufs=10) as ip,
            tc.tile_pool(name="xp", bufs=2) as xp,
            tc.tile_pool(name="sp", bufs=4) as sp,
            tc.tile_pool(name="fhp", bufs=3) as fhp,
            tc.tile_pool(name="r0p", bufs=2) as r0p,
            tc.tile_pool(name="bp", bufs=4) as bp,
            tc.tile_pool(name="pt", bufs=2, space="PSUM") as ptp,
            tc.tile_pool(name="po", bufs=2, space="PSUM") as pop,
            tc.tile_pool(name="dram", bufs=1, space="DRAM") as dp,
        ):
            # ---- constants ----
            ident = cp.tile([P, P], F32)
            make_identity(nc, ident[:])
            ones1 = cp.tile([1, P], F32)
            nc.gpsimd.memset(ones1[:], 1.0)
            if table_bf16:
                ones1t = cp.tile([1, P], TDT, name="ones1t")
                nc.gpsimd.memset(ones1t[:], 1.0)
            else:
                ones1t = ones1

            def load_w(name, src, kc):
                t = cp.tile([P, kc * D], F32, name=name)
                for kk in range(kc):
                    nc.sync.dma_start(out=t[:, kk * D:(kk + 1) * D],
                                      in_=src[kk * P:(kk + 1) * P, :])
                return t

            wfc_t = load_w("wfc_t", Wfc, KCN)
            wbc_t = load_w("wbc_t", Wbc, KCN)
            we_t = load_w("we_t", Wedge, KCE)

            def load_flat(name, src, shape, dt):
                t = cp.tile(shape, dt, name=name)
                nc.sync.dma_start(out=t[:], in_=src[:])
                return t

            bfc_t = load_flat("bfc_t", bfc, [1, D], F32)
            bbc_t = load_flat("bbc_t", bbc, [1, D], F32)
            be_t = load_flat("be_t", bedge, [1, D], F32)
            idx_dram = {"fw_adj": fw_adj_g, "bw_adj": bw_adj_g,
                        "fw_eid": fw_eid_g, "bw_eid": bw_eid_g,
                        "fw_dep": fw_dep_g, "bw_dep": bw_dep_g}
            # bf16 gather tiles free enough SBUF to keep all indices resident,
            # which removes 640 tiny HWDGE loads from the gather critical path
            idx_sb = None
            if table_bf16:
                idx_sb = {nm: load_flat(f"ti_{nm}", t, [P, t.shape[1]], I16)
                          for nm, t in idx_dram.items()}

            def load_blocked(name, src, nb, w, dt):
                tt = cp.tile([P, nb * w], dt, name=name)
                for b in range(nb):
                    nc.sync.dma_start(out=tt[:, b * w:(b + 1) * w],
                                      in_=src[b * P:(b + 1) * P, :])
                return tt

            rc_t = {nm: load_blocked(f"t_{nm}", t, nb, 1, F32)
                    for nm, t, nb in (("rcn_fw", rcn_fw, NB), ("rcn_bw", rcn_bw, NB),
                                      ("rce_fw", rce_fw, EB), ("rce_bw", rce_bw, EB),
                                      ("ca_fw", ca_fw, NB), ("ca_bw", ca_bw, NB),
                                      ("ce_fw", ce_fw, NB), ("ce_bw", ce_bw, NB),
                                      ("cd_fw", cd_fw, EB), ("cd_bw", cd_bw, EB))}

            # ---- DRAM tables (internal) ----
            def mk_tab(name, rows):
                return dp.tile([rows, D], TDT, addr_space="Shared", name=name)

            fw_tabA = mk_tab("fw_tabA", N)
            fw_tabB = mk_tab("fw_tabB", N)
            bw_tabA = mk_tab("bw_tabA", N)
            bw_tabB = mk_tab("bw_tabB", N)
            e_tabA = mk_tab("e_tabA", E)
            e_tabB = mk_tab("e_tabB", E)
            fw_sh = dp.tile([NS, D], F32, name="fw_sh")
            bw_sh = dp.tile([NS, D], F32, name="bw_sh")
            e_sh = dp.tile([ES, D], F32, name="e_sh")
            if table_bf16:
                fw_shB = dp.tile([NS, D], TDT, name="fw_shB")
                bw_shB = dp.tile([NS, D], TDT, name="bw_shB")
                e_shB = dp.tile([ES, D], TDT, name="e_shB")
            else:
                fw_shB, bw_shB, e_shB = fw_sh, bw_sh, e_sh

            fw_ntabs = [fw_tab0, fw_tabA, fw_tabB]
            bw_ntabs = [bw_tab0, bw_tabA, bw_tabB]
            e_tabs = [e_tab0, e_tabA, e_tabB]

            def bcast0(tab):
                """[128, D] tile with every partition = tab row 0."""
                r0 = r0p.tile([1, D], TDT, name="r0", tag="r0")
                nc.sync.dma_start(out=r0[:], in_=tab[0:1, :])
                ps = ptp.tile([P, D], F32, name="ps_t", tag="ps_t")
                nc.tensor.matmul(out=ps[:], lhsT=ones1t[:], rhs=r0[:],
                                 start=True, stop=True)
                b = bp.tile([P, D], F32, name="b0", tag="b0")
                nc.vector.tensor_copy(out=b[:], in_=ps[:])
                return b

            QJ = 4  # j-columns per gather call
            qctr = [0]

            def gather_q(tab, idx_nm, b, coloff, nj):
                if idx_sb is not None:
                    isl = idx_sb[idx_nm]
                    isl_ap = isl[:, coloff:coloff + nj * 8]
                else:
                    t = ip.tile([P, nj * 8], I16, name="isl", tag="isl")
                    nc.sync.dma_start(
                        out=t[:], in_=idx_dram[idx_nm][:, coloff:coloff + nj * 8])
                    isl_ap = t[:]
                g = gp.tile([P, nj * D], TDT, name="g", tag="g")
                qctr[0] = (qctr[0] + 1) % 4
                nc.gpsimd.dma_gather(
                    out_ap=g[:].rearrange("p (t e) -> p t e", e=D),
                    in_ap=tab[:],
                    idxs_ap=isl_ap,
                    num_idxs=nj * P,
                    num_idxs_reg=nj * P,
                    elem_size=D,
                    queue_num=qctr[0],
                )
                return g

            ADD = mybir.AluOpType.add

            def mean_sbuf(gtiles, rc, b, corrs):
                """sm[128,D] = rc_b * (sum_j G_j + sum_i corr_i * B0_i), on DVE.

                Sums each gather tile in place (pairwise tree), accumulates
                into sm, applies padding corrections and the 1/cnt scale.
                """
                sm = sp.tile([P, D], F32, name="sm", tag="sm")
                (c0, B00) = corrs[0]
                nc.vector.tensor_scalar_mul(sm[:], B00[:], c0[:, b:b + 1])
                for (cx, B0x) in corrs[1:]:
                    ct = sp.tile([P, D], F32, name="ct", tag="ct")
                    nc.vector.tensor_scalar_mul(ct[:], B0x[:], cx[:, b:b + 1])
                    nc.vector.tensor_tensor(out=sm[:], in0=sm[:], in1=ct[:], op=ADD)
                for g in gtiles:
                    if table_bf16:
                        tq = sp.tile([P, 2 * D], F32, name="tq", tag="tq")
                        nc.vector.tensor_tensor(
                            out=tq[:], in0=g[:, 0:2 * D], in1=g[:, 2 * D:4 * D], op=ADD)
                        nc.vector.tensor_tensor(
                            out=tq[:, 0:D], in0=tq[:, 0:D], in1=tq[:, D:2 * D], op=ADD)
                        nc.vector.tensor_tensor(out=sm[:], in0=sm[:], in1=tq[:, 0:D], op=ADD)
                    else:
                        nc.vector.tensor_tensor(
                            out=g[:, 0:2 * D], in0=g[:, 0:2 * D], in1=g[:, 2 * D:4 * D], op=ADD)
                        nc.vector.tensor_tensor(
                            out=g[:, 0:D], in0=g[:, 0:D], in1=g[:, D:2 * D], op=ADD)
                        nc.vector.tensor_tensor(out=sm[:], in0=sm[:], in1=g[:, 0:D], op=ADD)
                nc.vector.tensor_scalar_mul(sm[:], sm[:], rc[:, b:b + 1])
                return sm

            def transpose_into(xT, cbase, src_sb):
                pt = ptp.tile([P, DC * P], F32, name="ps_t")
                for c in range(DC):
                    nc.tensor.transpose(
                        out=pt[:, c * P:(c + 1) * P], in_=src_sb[:, c * P:(c + 1) * P],
                        identity=ident[:],
                    )
                nc.vector.tensor_copy(
                    out=xT[:, cbase * P:(cbase + DC) * P], in_=pt[:],
                )

            def linear(xT, kc, w_t, b_row, relu, out_sb):
                ps = pop.tile([P, D], F32, name="ps_o")
                for kk in range(kc):
                    nc.tensor.matmul(
                        out=ps[:], lhsT=xT[:, kk * P:(kk + 1) * P],
                        rhs=w_t[:, kk * D:(kk + 1) * D],
                        start=(kk == 0), stop=False,
                    )
                nc.tensor.matmul(
                    out=ps[:], lhsT=ones1[:], rhs=b_row[:], start=False, stop=True,
                )
                fn = (mybir.ActivationFunctionType.Relu if relu
                      else mybir.ActivationFunctionType.Copy)
                nc.scalar.activation(out=out_sb[:], in_=ps[:], func=fn)

            def node_block(k, b, ntab, etab, own_src, a_nm, e_nm, rc_nm,
                           ca_nm, ce_nm, w_t, b_row, dst, dstB, nB0, eB0):
                relu = (k < K - 1)
                nq = DEG // QJ
                gts = []
                for h in range(nq):
                    gts.append(gather_q(ntab, a_nm, b, b * NCOL + h * QJ * 8, QJ))
                for h in range(nq):
                    gts.append(gather_q(etab, e_nm, b, b * NCOL + h * QJ * 8, QJ))
                nf = sp.tile([P, D], F32, name="nf", tag="nf")
                nc.sync.dma_start(out=nf[:], in_=own_src[b * P:(b + 1) * P, :])

                sm = mean_sbuf(gts, rc_t[rc_nm], b,
                               [(rc_t[ca_nm], nB0), (rc_t[ce_nm], eB0)])

                xT = xp.tile([P, KCN * P], F32, name="xT", tag="xT")
                transpose_into(xT, 0, nf)
                transpose_into(xT, DC, sm)

                fh = fhp.tile([P, D], F32, name="fh", tag="fh")
                linear(xT, KCN, w_t, b_row, relu, fh)
                nc.sync.dma_start(out=dst[b * P:(b + 1) * P, :], in_=fh[:])
                if table_bf16 and dstB is not None:
                    fhb = fhp.tile([P, D], TDT, name="fhb", tag="fhb")
                    nc.vector.tensor_copy(out=fhb[:], in_=fh[:])
                    nc.sync.dma_start(out=dstB[b * P:(b + 1) * P, :], in_=fhb[:])

            def edge_block(u, b, fw_nt, bw_nt, own_src, fB0, bB0):
                eq = DEP // QJ
                gf = [gather_q(fw_nt, "fw_dep", b, b * ECOL + h * QJ * 8, QJ)
                      for h in range(eq)]
                gb = [gather_q(bw_nt, "bw_dep", b, b * ECOL + h * QJ * 8, QJ)
                      for h in range(eq)]
                eo = sp.tile([P, D], F32, name="eo", tag="nf")
                nc.sync.dma_start(out=eo[:], in_=own_src[b * P:(b + 1) * P, :])

                smf = mean_sbuf(gf, rc_t["rce_fw"], b, [(rc_t["cd_fw"], fB0)])
                smb = mean_sbuf(gb, rc_t["rce_bw"], b, [(rc_t["cd_bw"], bB0)])

                xT = xp.tile([P, KCE * P], F32, name="xTe", tag="xT")
                transpose_into(xT, 0, eo)
                transpose_into(xT, DC, smf)
                transpose_into(xT, 2 * DC, smb)

                es = fhp.tile([P, D], F32, name="es", tag="fh")
                linear(xT, KCE, we_t, be_t, True, es)
                nc.sync.dma_start(out=e_sh[b * P:(b + 1) * P, :], in_=es[:])
                if table_bf16:
                    esb = fhp.tile([P, D], TDT, name="esb", tag="fhb")
                    nc.vector.tensor_copy(out=esb[:], in_=es[:])
                    nc.sync.dma_start(out=e_shB[b * P:(b + 1) * P, :], in_=esb[:])

            def allgather(src, dsttab):
                nc.gpsimd.collective_compute(
                    "AllGather", mybir.AluOpType.bypass, replica_groups=rg,
                    ins=[src[:]], outs=[dsttab[:]],
                )

            for k in range(K):
                last = (k == K - 1)
                fw_dst = fw_sh if not last else fw_out
                bw_dst = bw_sh if not last else bw_out
                fw_ownsrc = fw_own0 if k == 0 else fw_sh
                bw_ownsrc = bw_own0 if k == 0 else bw_sh
                fB0 = bcast0(fw_ntabs[k])
                bB0 = bcast0(bw_ntabs[k])
                eB0 = bcast0(e_tabs[k])
                for b in range(NB):
                    node_block(k, b, fw_ntabs[k], e_tabs[k], fw_ownsrc,
                               "fw_adj", "fw_eid", "rcn_fw", "ca_fw", "ce_fw",
                               wfc_t, bfc_t, fw_dst,
                               fw_shB if not last else None, fB0, eB0)
                if not last:
                    allgather(fw_shB, fw_ntabs[k + 1])
                for b in range(NB):
                    node_block(k, b, bw_ntabs[k], e_tabs[k], bw_ownsrc,
                               "bw_adj", "bw_eid", "rcn_bw", "ca_bw", "ce_bw",
                               wbc_t, bbc_t, bw_dst,
                               bw_shB if not last else None, bB0, eB0)
                if not last:
                    allgather(bw_shB, bw_ntabs[k + 1])
                    e_ownsrc = e_own0 if k == 0 else e_sh
                    fB0e = bcast0(fw_ntabs[k + 1])
                    bB0e = bcast0(bw_ntabs[k + 1])
                    for b in range(EB):
                        edge_block(k, b, fw_ntabs[k + 1], bw_ntabs[k + 1],
                                   e_ownsrc, fB0e, bB0e)
                    allgather(e_shB, e_tabs[k + 1])

    nc.compile()
    return nc


def _pack_idx(idx, nb, w):
    """[nb*128, w] int -> [128, nb*w*8] int16 wrapped dma_gather layout."""
    out = np.empty((P, nb * w * 8), np.int16)
    c = w * 8
    for b in range(nb):
        lst = idx[b * P:(b + 1) * P].T.reshape(-1)      # i = j*128 + p
        wrapped = lst.reshape(-1, 16).T                  # [16, w*8]
        out[:, b * c:(b + 1) * c] = np.tile(wrapped, (8, 1))
    return out


def prep_inputs(cfg: Cfg, inputs: dict, table_bf16=False):
    N, E, D, DEG, DEP, CORES = cfg.N, cfg.E, cfg.D, cfg.DEG, cfg.DEP, cfg.CORES
    NS, ES, NB, EB = cfg.NS, cfg.ES, cfg.NB, cfg.EB
    f32 = np.float32

    fw = np.ascontiguousarray(np.asarray(inputs["fw_input"], f32))
    bw = np.ascontiguousarray(np.asarray(inputs["bw_input"], f32))
    ee = np.ascontiguousarray(np.asarray(inputs["edge_embs"], f32))
    if table_bf16:
        import ml_dtypes
        tdt = ml_dtypes.bfloat16
        fw_tab, bw_tab, ee_tab = fw.astype(tdt), bw.astype(tdt), ee.astype(tdt)
    else:
        fw_tab, bw_tab, ee_tab = fw, bw, ee

    idxs = {k: np.asarray(inputs[k], np.int64) for k in
            ("fw_adj", "bw_adj", "fw_edgeid", "bw_edgeid",
             "fw_edgedep", "bw_edgedep")}
    r0 = {k: np.where(v < 0, 0, v) for k, v in idxs.items()}
    deg = {k: (v >= 0).sum(1) for k, v in idxs.items()}

    cn_fw = deg["fw_adj"] + deg["fw_edgeid"]
    cn_bw = deg["bw_adj"] + deg["bw_edgeid"]

    def rec(c):
        with np.errstate(divide="ignore"):
            return (1.0 / c.astype(f32)).astype(f32)

    rcn_fw_f = rec(cn_fw)
    rcn_bw_f = rec(cn_bw)
    rce_fw_f = rec(deg["fw_edgedep"])
    rce_bw_f = rec(deg["bw_edgedep"])

    ca_fw_f = (-(DEG - deg["fw_adj"])).astype(f32)
    ce_fw_f = (-(DEG - deg["fw_edgeid"])).astype(f32)
    ca_bw_f = (-(DEG - deg["bw_adj"])).astype(f32)
    ce_bw_f = (-(DEG - deg["bw_edgeid"])).astype(f32)
    cd_fw_f = (-(DEP - deg["fw_edgedep"])).astype(f32)
    cd_bw_f = (-(DEP - deg["bw_edgedep"])).astype(f32)

    Wfc = np.ascontiguousarray(np.asarray(inputs["Wfc"], f32))
    Wbc = np.ascontiguousarray(np.asarray(inputs["Wbc"], f32))
    Wedge = np.ascontiguousarray(np.asarray(inputs["Wedge"], f32))
    bfc = np.asarray(inputs["bfc"], f32).reshape(1, D)
    bbc = np.asarray(inputs["bbc"], f32).reshape(1, D)
    bedge = np.asarray(inputs["bedge"], f32).reshape(1, D)

    in_maps = []
    for c in range(CORES):
        nsl = slice(c * NS, (c + 1) * NS)
        esl = slice(c * ES, (c + 1) * ES)
        in_maps.append({
            "fw_tab0": fw_tab, "bw_tab0": bw_tab, "e_tab0": ee_tab,
            "fw_own0": fw[nsl].copy(), "bw_own0": bw[nsl].copy(),
            "e_own0": ee[esl].copy(),
            "fw_adj_g": _pack_idx(r0["fw_adj"][nsl], NB, DEG),
            "bw_adj_g": _pack_idx(r0["bw_adj"][nsl], NB, DEG),
            "fw_eid_g": _pack_idx(r0["fw_edgeid"][nsl], NB, DEG),
            "bw_eid_g": _pack_idx(r0["bw_edgeid"][nsl], NB, DEG),
            "fw_dep_g": _pack_idx(r0["fw_edgedep"][esl], EB, DEP),
            "bw_dep_g": _pack_idx(r0["bw_edgedep"][esl], EB, DEP),
            "rcn_fw": rcn_fw_f[nsl, None].copy(), "rcn_bw": rcn_bw_f[nsl, None].copy(),
            "rce_fw": rce_fw_f[esl, None].copy(), "rce_bw": rce_bw_f[esl, None].copy(),
            "ca_fw": ca_fw_f[nsl, None].copy(), "ca_bw": ca_bw_f[nsl, None].copy(),
            "ce_fw": ce_fw_f[nsl, None].copy(), "ce_bw": ce_bw_f[nsl, None].copy(),
            "cd_fw": cd_fw_f[esl, None].copy(), "cd_bw": cd_bw_f[esl, None].copy(),
            "Wfc": Wfc, "Wbc": Wbc, "Wedge": Wedge,
            "bfc": bfc, "bbc": bbc, "bedge": bedge,
        })
    return in_maps


def assemble_outputs(cfg: Cfg, results):
    fw = np.concatenate([results[c]["fw_out"] for c in range(cfg.CORES)], axis=0)
    bw = np.concatenate([results[c]["bw_out"] for c in range(cfg.CORES)], axis=0)
    return fw, bw


# ======================= self-contained runner =======================
import os as _os
import types as _types


def _install_axon_prof():
    """Provide antenv.axon_hooks + NTFF hook so trace=True works under axon."""
    name = "antenv.axon_hooks"
    if name in sys.modules:
        return True
    try:
        mod = _types.ModuleType(name)
        mod._hook = None
        mod.set_axon_ntff_profile_hook = lambda h: setattr(mod, "_hook", h)
        mod.get_axon_ntff_profile_hook = lambda: mod._hook
        sys.modules[name] = mod
        import antenv
        antenv.axon_hooks = mod
        from trn_agent_boot.trn_boot import _ntff_profile_via_ctypes
        mod.set_axon_ntff_profile_hook(
            _ntff_profile_via_ctypes('/opt/axon/libaxon_pjrt.so'))
        return True
    except Exception:
        sys.modules.pop(name, None)
        return False


_CACHE = {}
LAST_EXEC_NS = None
LAST_PROFILE = None


def kernel(**inputs):
    """Full-input GNN forward on 8 TRN2 NeuronCores. Returns (fw, bw)."""
    global LAST_EXEC_NS, LAST_PROFILE
    from concourse.bass_utils import run_bass_kernel_spmd

    cfg = Cfg()
    bf16 = _os.environ.get("GNN_F32", "0") != "1"
    if "nc" not in _CACHE:
        _CACHE["nc"] = build(cfg, table_bf16=bf16)
    nc = _CACHE["nc"]

    in_maps = prep_inputs(cfg, inputs, table_bf16=bf16)

    profile = _os.environ.get("GNN_PROFILE", "0") == "1"
    if profile:
        profile = _install_axon_prof()
    res = run_bass_kernel_spmd(nc, in_maps, core_ids=list(range(cfg.CORES)),
                               trace=profile)
    LAST_EXEC_NS = res.exec_time_ns
    LAST_PROFILE = res.profile_json
    if res.instructions_and_trace is not None:
        try:
            print("trace:", res.instructions_and_trace[1])
        except Exception:
            pass
    return assemble_outputs(cfg, res.results)

